# revision 13
# baseline (speedup 1.0000x reference)
"""DeepSet-equivariant layer on 8 TRN2 NeuronCores.

Math (reference):
    y = x @ w1 + (colsum(x) @ w2) / n + bias        x: (n, 128)

Distribution: shard x and y along the set dimension n across the 8 cores;
each core computes its local column-sum; the 8 local sums are exchanged
with ONE remote_dma_broadcast per core (SBUF -> 7 peers' SBUF, a few us,
bypassing the ~90us ncfw collective path); w1/w2/bias are replicated.

Layout: each core receives its shard pre-transposed as xT (128 = d_in
partitions, rows free) in bf16 and returns yT in bf16 (host up-casts).
bf16 I/O halves both HBM phases; against the 2e-2 harness tolerance the
~2e-3 quantization error is comfortably safe.

Schedule per core:
  - dummy CC AllReduce issued first: NEFFs containing a collective get a
    rank-coordinated launch (without one, core starts skew by ms).
    Nothing waits on it.
  - phase 1 streams xT into a resident SBUF buffer; per chunk a column-
    sum partial runs on DVE (even) / ACT accum (odd).
  - the local colsum lands in slot 0 of a [128,16] gather buffer; one
    remote_dma_broadcast sends it to slot 8+my_id on the 7 peers (XOR
    routing, slot = sender id, each slot written exactly once). The
    trigger is gated on the colsum via an explicit semaphore (Tile
    treats remote-DMA preps as user-synced: no automatic data deps).
  - after 14 remote-sem increments (7 senders x 2) the slots are
    reduced, t = (global colsum @ w2)/n + bias.
  - phase 2: matmul from resident x, +t fused into the PSUM drain
    (alternating DVE/ACT), bf16 out.

Tile's single-core scheduling sim cannot model remote arrivals (it
would deadlock on the wait), so the two protocol waits are emitted
after the TileContext and spliced into engine-queue position by direct
BIR list surgery.
"""

import numpy as np
import ml_dtypes

import concourse.bass as bass
import concourse.tile as tile
from concourse import bacc, mybir
from concourse.bass_utils import run_bass_kernel_spmd

N_CORES = 8
D = 128                 # d_in == d_out
N_ROWS = 200000         # full set size
R = 25088               # padded rows per core: 8 * 25088 = 200704 >= 200000
IN_CHUNK = 2048         # columns per input DMA chunk (512 KiB bf16)
MM_N = 512              # moving-operand free dim per matmul (ISA cap)
PS_N = 2048             # columns per PSUM tile (4 banks fp32)
OUT_CHUNK = 4096        # columns per output buffer (1 MiB bf16)

F32 = mybir.dt.float32
BF16 = mybir.dt.bfloat16
NP_BF16 = ml_dtypes.bfloat16

GATHER_SLOTS = 16       # slot 0: local colsum; slots 8+sender: remote


# input chunk widths: large early (DMA efficiency), small late (short
# colsum tail before the cross-core exchange fires)
IN_WIDTHS = [8192, 8192, 4096, 2048, 1024, 768, 512, 256]
assert sum(IN_WIDTHS) == R


def _split(r, step):
    out = []
    c0 = 0
    while c0 < r:
        cw = min(step, r - c0)
        out.append((c0, cw))
        c0 += cw
    return out


def _split_widths(widths):
    out = []
    c0 = 0
    for w in widths:
        out.append((c0, w))
        c0 += w
    return out


def _move_before(nc, inst, target):
    """Move a post-TileContext instruction directly before `target` in the
    block that holds it (engine dispatch follows list order per engine)."""
    src = dst = None
    for bb in nc.m.functions[0].blocks:
        names = [i.name for i in bb.instructions]
        if inst.name in names:
            src = bb
        if target.name in names:
            dst = bb
    assert src is not None and dst is not None
    src.instructions.remove(inst)
    dst.instructions.insert(dst.instructions.index(target), inst)


def _move_after(nc, inst, target):
    src = dst = None
    for bb in nc.m.functions[0].blocks:
        names = [i.name for i in bb.instructions]
        if inst.name in names:
            src = bb
        if target.name in names:
            dst = bb
    assert src is not None and dst is not None
    src.instructions.remove(inst)
    dst.instructions.insert(dst.instructions.index(target) + 1, inst)


def build_nc(r: int, n_total: int):
    in_chunks = _split_widths(IN_WIDTHS) if r == R else _split(r, IN_CHUNK)
    out_chunks = _split(r, OUT_CHUNK)

    nc = bacc.Bacc(
        "TRN2",
        target_bir_lowering=False,
        debug=False,
        num_devices=N_CORES,
    )

    xt = nc.declare_dram_parameter("xt", [D, r], BF16, isOutput=False)
    w1 = nc.declare_dram_parameter("w1", [D, D], BF16, isOutput=False)
    w2 = nc.declare_dram_parameter("w2", [D, D], F32, isOutput=False)
    bias_c = nc.declare_dram_parameter("bias_c", [D, 1], F32, isOutput=False)
    out = nc.declare_dram_parameter("out", [D, r], BF16, isOutput=True)

    # Dummy collective for rank-coordinated launch; nothing waits on it.
    ccw_in = nc.dram_tensor("ccw_in", [D, 1], F32)
    ccw_out = nc.dram_tensor("ccw_out", [D, 1], F32, addr_space="Shared")
    warm_sem = nc.alloc_semaphore("warm_cc")
    nc.gpsimd.collective_compute(
        "AllReduce",
        mybir.AluOpType.add,
        replica_groups=[list(range(N_CORES))],
        ins=[ccw_in.ap().opt()],
        outs=[ccw_out.ap().opt()],
    ).then_inc(warm_sem)

    gsem = nc.alloc_semaphore("gather_sem")
    lsem = nc.alloc_semaphore("rdma_local")
    cs_sem = nc.alloc_semaphore("cs_done")

    # Fixed-address gather buffer (remote cores write slots 8..15).
    gather_sb = nc.alloc_sbuf_tensor("gather_sb", [D, GATHER_SLOTS], F32)

    with tile.TileContext(nc) as tc:
        with (
            tc.tile_pool(name="const", bufs=1) as const_pool,
            tc.tile_pool(name="xres", bufs=1) as xres_pool,
            tc.tile_pool(name="obuf", bufs=4) as obuf_pool,
            tc.tile_pool(name="mm", bufs=2, space=bass.MemorySpace.PSUM) as mm_pool,
        ):
            w1_sb = const_pool.tile([D, D], BF16)
            w2_sb = const_pool.tile([D, D], F32)
            bias_sb = const_pool.tile([D, 1], F32)

            nc.gpsimd.memset(gather_sb[:, :], 0.0)

            # broadcast destination slot offset: 8 + my core id (elements)
            off_gp = nc.gpsimd.alloc_register("slot_off")
            nc.gpsimd.reg_load(off_gp, nc.partition_id_tensor[0:1, 0:1])
            nc.gpsimd.reg_add(off_gp, off_gp, 8)

            # one broadcast: my slot 0 -> peers' slot 8+my_id (self = None)
            slot_out = bass.AP(gather_sb, off_gp, [[GATHER_SLOTS, D], [1, 1]])
            rdests = [None] + [(0, k) for k in range(1, N_CORES)]
            nc.gpsimd.remote_dma_broadcast(
                slot_out,
                gather_sb[:, 0:1],
                gsem,
                lsem,
                rdests=rdests,
            )

            # phase 1: stream xT into resident SBUF; per-chunk colsum
            x_sb = xres_pool.tile([D, r], BF16)
            n_in = len(in_chunks)
            cs_parts = const_pool.tile([D, n_in], F32)
            trash = const_pool.tile([D, max(cw for _, cw in in_chunks)], BF16)

            for c, (c0, cw) in enumerate(in_chunks):
                dma_eng = nc.sync if c % 2 == 0 else nc.scalar
                dma_eng.dma_start(x_sb[:, c0 : c0 + cw], xt[:, c0 : c0 + cw])
                if c % 2 == 0 or c == n_in - 1:
                    nc.vector.reduce_sum(
                        cs_parts[:, c : c + 1],
                        x_sb[:, c0 : c0 + cw],
                        axis=mybir.AxisListType.X,
                    )
                else:
                    nc.scalar.activation(
                        trash[:, :cw],
                        x_sb[:, c0 : c0 + cw],
                        mybir.ActivationFunctionType.Copy,
                        accum_out=cs_parts[:, c : c + 1],
                    )

            nc.sync.dma_start(w1_sb[:], w1[:, :])
            nc.sync.dma_start(w2_sb[:], w2[:, :])
            nc.sync.dma_start(bias_sb[:], bias_c[:, :])

            # local colsum -> gather slot 0, gate + fire the exchange
            cs_reduce = nc.vector.reduce_sum(
                gather_sb[:, 0:1], cs_parts[:], axis=mybir.AxisListType.X
            )
            trig = nc.gpsimd.trigger_dma(
                count=None, signals_writable=[gather_sb[:, :]]
            )

            # global colsum -> transmit column t
            gcs = const_pool.tile([D, 1], F32)
            t_sb = const_pool.tile([D, 1], F32)
            gcs_reduce = nc.vector.reduce_sum(
                gcs[:], gather_sb[:, :], axis=mybir.AxisListType.X
            )
            t_ps = mm_pool.tile([D, PS_N], F32, tag="ps")
            nc.tensor.matmul(t_ps[:, :1], w2_sb[:], gcs[:])
            nc.vector.tensor_scalar(
                out=t_sb[:],
                in0=t_ps[:, :1],
                scalar1=1.0 / float(n_total),
                scalar2=bias_sb[:],
                op0=mybir.AluOpType.mult,
                op1=mybir.AluOpType.add,
            )

            # phase 2: matmul from resident x, fuse +t into the PSUM drain
            drain_flip = 0
            for c, (c0, cw) in enumerate(out_chunks):
                ob = obuf_pool.tile([D, OUT_CHUNK], BF16)
                p0 = 0
                while p0 < cw:
                    pw = min(PS_N, cw - p0)
                    ps = mm_pool.tile([D, PS_N], F32, tag="ps")
                    s0 = 0
                    while s0 < pw:
                        sw = min(MM_N, pw - s0)
                        nc.tensor.matmul(
                            ps[:, s0 : s0 + sw],
                            w1_sb[:],
                            x_sb[:, c0 + p0 + s0 : c0 + p0 + s0 + sw],
                        )
                        s0 += sw
                    if drain_flip % 2 == 0:
                        nc.vector.tensor_scalar(
                            out=ob[:, p0 : p0 + pw],
                            in0=ps[:, :pw],
                            scalar1=t_sb[:],
                            scalar2=None,
                            op0=mybir.AluOpType.add,
                        )
                    else:
                        nc.scalar.activation(
                            ob[:, p0 : p0 + pw],
                            ps[:, :pw],
                            mybir.ActivationFunctionType.Identity,
                            bias=t_sb[:],
                            scale=1.0,
                        )
                    drain_flip += 1
                    p0 += pw
                (nc.sync if c % 2 == 0 else nc.scalar).dma_start(
                    out[:, c0 : c0 + cw], ob[:, :cw]
                )

    # Protocol signal + waits, invisible to Tile's scheduling sim:
    #  - cs_sem inc right after the colsum reduce on DVE
    #  - trigger must not fire before the local colsum is written
    #  - the gather reduce must not read before all 7 remote slots landed
    inc_cs = nc.vector.sem_inc(cs_sem, 1)
    _move_after(nc, inc_cs.ins, cs_reduce.ins)
    w_cs = nc.gpsimd.wait_ge(cs_sem, 1)
    _move_before(nc, w_cs.ins, trig.ins)
    w_arr = nc.vector.wait_ge(gsem, 14)
    _move_before(nc, w_arr.ins, gcs_reduce.ins)

    nc.compile()
    return nc


_nc_cache: dict = {}


def _get_nc(r: int, n_total: int):
    key = (r, n_total)
    if key not in _nc_cache:
        _nc_cache[key] = build_nc(r, n_total)
    return _nc_cache[key]


LAST_RESULTS = None


def _execute(x, w1, w2, bias, r, trace=False, tmpdir=None, trace_cores=None):
    global LAST_RESULTS
    x = np.ascontiguousarray(np.asarray(x, dtype=np.float32))
    w1 = np.ascontiguousarray(np.asarray(w1, dtype=np.float32))
    w2 = np.ascontiguousarray(np.asarray(w2, dtype=np.float32))
    bias = np.asarray(bias, dtype=np.float32)
    n, d = x.shape
    assert d == D and r * N_CORES >= n

    xp = np.zeros((N_CORES * r, d), dtype=np.float32)
    xp[:n] = x
    # (8, r, d) -> (8, d, r) pre-transposed bf16 shards
    xts = np.ascontiguousarray(
        xp.reshape(N_CORES, r, d).transpose(0, 2, 1)
    ).astype(NP_BF16)
    w1_bf = w1.astype(NP_BF16)
    bias_col = np.ascontiguousarray(bias.reshape(1, d).T)

    in_maps = [
        {"xt": xts[i], "w1": w1_bf, "w2": w2, "bias_c": bias_col}
        for i in range(N_CORES)
    ]

    nc = _get_nc(r, n)
    kwargs = {}
    if trace:
        kwargs.update(trace=True, tmpdir=tmpdir)
        if trace_cores is not None:
            kwargs.update(trace_cores=trace_cores)
    res = run_bass_kernel_spmd(nc, in_maps, core_ids=list(range(N_CORES)), **kwargs)
    LAST_RESULTS = res

    yts = [res.results[i]["out"] for i in range(N_CORES)]  # each (D, r) bf16
    y = np.concatenate([yt.T for yt in yts], axis=0)[:n].astype(np.float32)
    return np.ascontiguousarray(y)


def kernel(x, w1, w2, bias):
    return _execute(x, w1, w2, bias, R)
